# revision 7
# baseline (speedup 1.0000x reference)
"""Trainium2 Bass kernel for nn_Copy61CQCNN (MLP + 4-qubit statevector block).

Strategy
--------
* Data-parallel over batch: 131072 rows -> 8 cores x 16384 rows.
* The whole network is reformulated as a pure matmul chain in "layout A"
  (features on partitions, batch on the free dim):

    h1 = relu(W1 x + b1)                       K=252 (split 128+124), M=120
    h2 = relu(W2' h1 + b2')                    (BN1 folded)           M=60
    h3 = relu(W3' h2 + b3')                    (BN2 folded)           M=28
    phi = W4'' h3          (duplicated rows: cos block & sin block)   M=2*nf
    feats = Sin(phi + b4'')   one ACT op (cos via +pi/2 bias)
    h5 = relu(W5'' feats + b5'')               (harmonic coeffs folded) M=56
    h6 = relu(W6' h5 + b6')                    (BN5 folded)           M=112
    h7 = relu(W7' h6 + b7')                    (BN6 folded)           M=56
    y  = W8 h7 + b8                                                   M=36

  The quantum block is EXACT as a 41-frequency trigonometric polynomial:
  z_i(theta) = sum_f a_if cos(f.theta) + b_if sin(f.theta), f in {0,+-1}^4
  (canonical half-space).  Coefficients are fit at runtime from params['qw']
  by least squares in float64 (residual ~1e-15).

* x is transposed host-side per shard so all DMA is contiguous; matmuls use
  float32r (full-rate fp32 streaming on the PE).
"""

import sys

sys.path.insert(0, "/opt/trn_rl_repo")

import itertools
import math
from contextlib import ExitStack

import numpy as np

import concourse.bass as bass  # noqa: E402
import concourse.tile as tile  # noqa: E402
from concourse import bacc, mybir  # noqa: E402
from concourse.bass_utils import run_bass_kernel_spmd  # noqa: E402

F32 = mybir.dt.float32
F32R = mybir.dt.float32r
AF = mybir.ActivationFunctionType
ALU = mybir.AluOpType

B_TOTAL = 131072
D_IN = 252
D_OUT = 36
N_CORES = 8
B_CORE = B_TOTAL // N_CORES  # 16384
SUPER = 2048                 # columns handled per DMA round
BT = 512                     # matmul free dim (one PSUM bank of fp32)

_CNOT_PAIRS = [(0, 1), (1, 2), (2, 3), (3, 0)]
_BN_EPS = 1e-5


# ----------------------------------------------------------------------------
# host-side math: quantum block -> trigonometric polynomial
# ----------------------------------------------------------------------------

def _quantum_np(xq, w):
    B = xq.shape[0]
    s = np.zeros((B, 16), dtype=np.complex128)
    s[:, 0] = 1.0
    s = s.reshape(B, 2, 2, 2, 2)

    def apply_1q(s, g, wire):
        s = np.moveaxis(s, wire + 1, -1)
        if g.ndim == 3:
            s = np.einsum("zab,z...b->z...a", g, s)
        else:
            s = np.einsum("ab,...b->...a", g, s)
        return np.moveaxis(s, -1, wire + 1)

    def apply_rz(s, phi, wire):
        f = np.stack([np.exp(-0.5j * phi), np.exp(0.5j * phi)])
        s = np.moveaxis(s, wire + 1, -1) * f
        return np.moveaxis(s, -1, wire + 1)

    def apply_cnot(s, c, t):
        s = np.moveaxis(s, [c + 1, t + 1], [-2, -1])
        s = np.stack([s[..., 0, :], s[..., 1, ::-1]], axis=-2)
        return np.moveaxis(s, [-2, -1], [c + 1, t + 1])

    half = xq * (np.pi / 4)
    c, si = np.cos(half), np.sin(half)
    for i in range(4):
        g = np.stack([np.stack([c[:, i], -si[:, i]], -1),
                      np.stack([si[:, i], c[:, i]], -1)], -2).astype(np.complex128)
        s = apply_1q(s, g, i)
    for l in range(2):
        for i in range(4):
            t = 0.5 * w[l, i, 0]
            ry = np.array([[np.cos(t), -np.sin(t)], [np.sin(t), np.cos(t)]],
                          dtype=np.complex128)
            s = apply_1q(s, ry, i)
            s = apply_rz(s, w[l, i, 1], i)
        for (cq, tq) in _CNOT_PAIRS:
            s = apply_cnot(s, cq, tq)
    p = s.real ** 2 + s.imag ** 2
    zs = []
    for i in range(4):
        axes = tuple(a for a in range(1, 5) if a != i + 1)
        m = p.sum(axis=axes)
        zs.append(m[:, 0] - m[:, 1])
    return np.stack(zs, axis=-1)


def _derive_harmonics(qw, trunc_tol=1e-7):
    """Fit z(theta) = Ccos cos(F theta) + Csin sin(F theta).

    Returns freqs [nf,4] (nonzero, truncated), Ccos/Csin [4,nf], c0 [4]
    (the zero-frequency constant).
    """
    freqs = []
    for f in itertools.product([-1, 0, 1], repeat=4):
        fa = np.array(f)
        nz = np.nonzero(fa)[0]
        if len(nz) == 0 or fa[nz[0]] > 0:
            freqs.append(fa)
    freqs = np.array(sorted(freqs, key=lambda f: (np.abs(f).sum(), list(f))),
                     dtype=np.float64)
    F = len(freqs)  # 41, zero freq included

    rng = np.random.RandomState(1234)
    theta = rng.uniform(-4, 4, size=(4096, 4))
    z = _quantum_np(theta * (2 / np.pi), np.asarray(qw, np.float64))
    X = np.concatenate([np.cos(theta @ freqs.T), np.sin(theta @ freqs.T)], axis=1)
    coef, *_ = np.linalg.lstsq(X, z, rcond=None)
    pred = X @ coef
    fit_err = np.abs(pred - z).max()
    assert fit_err < 1e-9, f"harmonic fit failed: {fit_err}"

    Ccos = coef[:F].T  # [4, F]
    Csin = coef[F:].T  # [4, F]

    zero_idx = int(np.where((freqs == 0).all(axis=1))[0][0])
    c0 = Ccos[:, zero_idx].copy()
    keep = [i for i in range(F) if i != zero_idx]
    mag = np.sqrt((Ccos ** 2).sum(0) + (Csin ** 2).sum(0))
    keep = [i for i in keep if mag[i] > trunc_tol]
    return freqs[keep], Ccos[:, keep], Csin[:, keep], c0


def _fold_params(params):
    """Fold BN layers + harmonic coefficients into a pure matmul chain.

    Returns dict of float64 lhsT weight blocks / bias vectors and nf.
    """
    p = {k: np.asarray(v, np.float64) for k, v in params.items()}
    freqs, Ccos, Csin, c0 = _derive_harmonics(p["qw"])
    nf = freqs.shape[0]

    g1 = p["g1"] / np.sqrt(1.0 + _BN_EPS)
    g2 = p["g2"] / np.sqrt(1.0 + _BN_EPS)
    g5 = p["g5"] / np.sqrt(1.0 + _BN_EPS)
    g6 = p["g6"] / np.sqrt(1.0 + _BN_EPS)

    W1, b1 = p["w1"], p["b1"]
    W2 = p["w2"] * g1[None, :]
    b2 = p["w2"] @ p["beta1"] + p["b2"]
    W3 = p["w3"] * g2[None, :]
    b3 = p["w3"] @ p["beta2"] + p["b3"]

    # phi rows: [cos-block; sin-block], each = (pi/2) F W4
    F_W4 = freqs @ p["w4"] * (np.pi / 2)        # [nf, 28]
    F_b4 = freqs @ p["b4"] * (np.pi / 2)        # [nf]
    W4d = np.concatenate([F_W4, F_W4], axis=0)  # [2nf, 28]
    b4d = np.concatenate([F_b4 + np.pi / 2, F_b4])  # cos rows first (+pi/2)

    # h5 = relu(W5 z + b5), z = Ccos ct + Csin st + c0
    W5e = np.concatenate([p["w5"] @ Ccos, p["w5"] @ Csin], axis=1)  # [56, 2nf]
    b5e = p["b5"] + p["w5"] @ c0

    W6 = p["w6"] * g5[None, :]
    b6 = p["w6"] @ p["beta5"] + p["b6"]
    W7 = p["w7"] * g6[None, :]
    b7 = p["w7"] @ p["beta6"] + p["b7"]
    W8, b8 = p["w8"], p["b8"]

    return {
        "W1": W1, "b1": b1, "W2": W2, "b2": b2, "W3": W3, "b3": b3,
        "W4d": W4d, "b4d": b4d, "W5e": W5e, "b5e": b5e,
        "W6": W6, "b6": b6, "W7": W7, "b7": b7, "W8": W8, "b8": b8,
        "nf": nf,
    }


def _host_forward(x, fp):
    """Numpy forward pass of the folded network (for phi-range check & debug)."""
    relu = lambda v: np.maximum(v, 0.0)
    h = relu(x @ fp["W1"].T + fp["b1"])
    h = relu(h @ fp["W2"].T + fp["b2"])
    h = relu(h @ fp["W3"].T + fp["b3"])
    phi = h @ fp["W4d"].T + fp["b4d"]
    feats = np.sin(phi)
    h = relu(feats @ fp["W5e"].T + fp["b5e"])
    h = relu(h @ fp["W6"].T + fp["b6"])
    h = relu(h @ fp["W7"].T + fp["b7"])
    return h @ fp["W8"].T + fp["b8"], phi


# ----------------------------------------------------------------------------
# weight packing
# ----------------------------------------------------------------------------

def _round_f32r(a):
    """Round fp32 to the TRN2 FP32R format (1s + 8e + 11m, RNE)."""
    u = np.ascontiguousarray(a, np.float32).view(np.uint32).astype(np.uint64)
    bias = 0x7FF + ((u >> 12) & 1)
    u = (u + bias) & 0xFFFFF000
    return u.astype(np.uint32).view(np.float32)


def _pack_weights(fp):
    nf = fp["nf"]
    nf2 = 2 * nf

    blocks = [  # (K, M, lhsT [K, M])
        ("w1a", fp["W1"][:, :128].T),            # [128, 120]
        ("w1b", fp["W1"][:, 128:].T),            # [124, 120]
        ("w2", fp["W2"].T),                      # [120, 60]
        ("w3", fp["W3"].T),                      # [60, 28]
        ("w4d", fp["W4d"].T),                    # [28, 2nf]
        ("w5", fp["W5e"].T),                     # [2nf, 56]
        ("w6", fp["W6"].T),                      # [56, 112]
        ("w7", fp["W7"].T),                      # [112, 56]
        ("w8", fp["W8"].T),                      # [56, 36]
    ]
    offs = {}
    col = 0
    for name, blk in blocks:
        offs[name] = (col, blk.shape[0], blk.shape[1])
        col += blk.shape[1]
    NW = col
    wpack = np.zeros((128, NW), np.float32)
    for name, blk in blocks:
        c0, K, M = offs[name]
        wpack[:K, c0:c0 + M] = blk.astype(np.float32)

    biases = [("b1", fp["b1"]), ("b2", fp["b2"]), ("b3", fp["b3"]),
              ("b4d", fp["b4d"]), ("b5", fp["b5e"]), ("b6", fp["b6"]),
              ("b7", fp["b7"]), ("b8", fp["b8"])]
    bpack = np.zeros((128, len(biases)), np.float32)
    boffs = {}
    for i, (name, b) in enumerate(biases):
        bpack[:len(b), i] = b.astype(np.float32)
        boffs[name] = i
    return wpack, bpack, offs, boffs, nf2


# ----------------------------------------------------------------------------
# bass program
# ----------------------------------------------------------------------------

def build_program(nf2, n_cols=B_CORE, offs=None, boffs=None, NW=None):
    """Build the SPMD Bass program.  n_cols = batch rows per core."""
    assert n_cols % SUPER == 0
    n_super = n_cols // SUPER
    n_sub = SUPER // BT

    nc = bacc.Bacc("TRN2", target_bir_lowering=False, debug=False,
                   enable_asserts=False, num_devices=N_CORES)

    xt = nc.dram_tensor("xt", [D_IN, n_cols], F32R, kind="ExternalInput").ap()
    wp = nc.dram_tensor("wpack", [128, NW], F32R, kind="ExternalInput").ap()
    bp = nc.dram_tensor("bpack", [128, 8], F32, kind="ExternalInput").ap()
    yt = nc.dram_tensor("yt", [D_OUT, n_cols], F32, kind="ExternalOutput").ap()

    def W(name):
        c0, K, M = offs[name]
        return None, c0, K, M

    with ExitStack() as ctx:
        tc = ctx.enter_context(tile.TileContext(nc))
        wpool = ctx.enter_context(tc.tile_pool(name="w", bufs=1))
        xpool = ctx.enter_context(tc.tile_pool(name="x", bufs=3))
        hpool = ctx.enter_context(tc.tile_pool(name="h", bufs=2))
        ypool = ctx.enter_context(tc.tile_pool(name="y", bufs=2))
        pspool = ctx.enter_context(tc.tile_pool(name="ps", bufs=6, space="PSUM"))

        wsb = wpool.tile([128, NW], F32R)
        nc.sync.dma_start(wsb[:], wp[:])
        bsb = wpool.tile([128, 8], F32)
        nc.sync.dma_start(bsb[:], bp[:])

        def wap(name):
            c0, K, M = offs[name]
            return wsb[0:K, c0:c0 + M]

        def bias(name, P):
            i = boffs[name]
            return bsb[0:P, i:i + 1]

        def mm(pool_tag, lhs_name, rhs_ap, extra=None):
            c0, K, M = offs[lhs_name]
            ps = pspool.tile([M, BT], F32, tag="ps")
            nc.tensor.matmul(ps[:], wap(lhs_name), rhs_ap,
                             start=True, stop=(extra is None))
            if extra is not None:
                nc.tensor.matmul(ps[:], wap(extra[0]), extra[1],
                                 start=False, stop=True)
            return ps

        def act_relu(ps, bname, P):
            h = hpool.tile([P, BT], F32R, tag=bname)
            nc.scalar.activation(h[:], ps[:], AF.Relu, bias=bias(bname, P))
            return h

        def dve_relu(ps, bname, P):
            h = hpool.tile([P, BT], F32R, tag=bname)
            nc.vector.tensor_scalar(h[:], ps[:], bias(bname, P), 0.0,
                                    op0=ALU.add, op1=ALU.max)
            return h

        for s in range(n_super):
            cs = s * SUPER
            xa = xpool.tile([128, SUPER], F32R, tag="xa")
            nc.sync.dma_start(xa[:], xt[0:128, cs:cs + SUPER])
            xb = xpool.tile([124, SUPER], F32R, tag="xb")
            nc.sync.dma_start(xb[:], xt[128:252, cs:cs + SUPER])
            yb = ypool.tile([D_OUT, SUPER], F32, tag="y")

            for j in range(n_sub):
                c = slice(j * BT, (j + 1) * BT)
                ps1 = mm("ps", "w1a", xa[:, c], extra=("w1b", xb[:, c]))
                h1 = act_relu(ps1, "b1", 120)
                ps2 = mm("ps", "w2", h1[:])
                h2 = dve_relu(ps2, "b2", 60)
                ps3 = mm("ps", "w3", h2[:])
                h3 = dve_relu(ps3, "b3", 28)
                ps4 = mm("ps", "w4d", h3[:])
                ft = hpool.tile([nf2, BT], F32R, tag="ft")
                nc.scalar.activation(ft[:], ps4[:], AF.Sin, bias=bias("b4d", nf2))
                ps5 = mm("ps", "w5", ft[:])
                h5 = dve_relu(ps5, "b5", 56)
                ps6 = mm("ps", "w6", h5[:])
                h6 = act_relu(ps6, "b6", 112)
                ps7 = mm("ps", "w7", h6[:])
                h7 = dve_relu(ps7, "b7", 56)
                ps8 = mm("ps", "w8", h7[:])
                nc.vector.tensor_scalar(yb[:, c], ps8[:], bias("b8", D_OUT), None,
                                        op0=ALU.add)

            nc.sync.dma_start(yt[:, cs:cs + SUPER], yb[:])

    nc.compile()
    return nc


# ----------------------------------------------------------------------------
# entry point
# ----------------------------------------------------------------------------

_PROG_CACHE = {}
LAST_EXEC_TIME_NS = None
LAST_RESULTS = None


def _get_program(nf2, n_cols, offs_key, offs, boffs, NW):
    key = (nf2, n_cols, offs_key)
    if key not in _PROG_CACHE:
        _PROG_CACHE[key] = build_program(nf2, n_cols, offs, boffs, NW)
    return _PROG_CACHE[key]


def kernel(x, params, trace=False):
    global LAST_EXEC_TIME_NS, LAST_RESULTS
    x = np.asarray(x, np.float32)
    assert x.shape == (B_TOTAL, D_IN), x.shape

    fp = _fold_params(params)
    wpack, bpack, offs, boffs, nf2 = _pack_weights(fp)
    NW = wpack.shape[1]

    # range guard for the Sin LUT ([-pi, pi] on the scalar engine): compute
    # exact phi range host-side (cheap BLAS pass over the folded fp32 net)
    fp32 = {k: (np.asarray(v, np.float32) if isinstance(v, np.ndarray) else v)
            for k, v in fp.items()}
    _, phi = _host_forward(x, fp32)
    phi_max = float(np.abs(phi).max())
    assert phi_max < 3.1, (
        f"phi range {phi_max} exceeds scalar-engine Sin domain; "
        "mod-based range reduction required")

    offs_key = tuple(sorted((k, v[0], v[1], v[2]) for k, v in offs.items()))
    nc = _get_program(nf2, B_CORE, offs_key, offs, boffs, NW)

    wpack = _round_f32r(wpack)
    in_maps = []
    for c in range(N_CORES):
        xs = _round_f32r(np.ascontiguousarray(x[c * B_CORE:(c + 1) * B_CORE].T))
        in_maps.append({"xt": xs, "wpack": wpack, "bpack": bpack})

    res = run_bass_kernel_spmd(nc, in_maps, core_ids=list(range(N_CORES)),
                               trace=trace)
    LAST_EXEC_TIME_NS = res.exec_time_ns
    LAST_RESULTS = res

    y = np.empty((B_TOTAL, D_OUT), np.float32)
    for c in range(N_CORES):
        y[c * B_CORE:(c + 1) * B_CORE] = res.results[c]["yt"].T
    return y


# revision 9
# speedup vs baseline: 24.9572x; 24.9572x over previous
"""Trainium2 Bass kernel for nn_Copy61CQCNN (MLP + 4-qubit statevector block).

Strategy
--------
* Data-parallel over batch: 131072 rows -> 8 cores x 16384 rows.
* The whole network is reformulated as a pure matmul chain in "layout A"
  (features on partitions, batch on the free dim):

    h1 = relu(W1 x + b1)                       K=252 (split 128+124), M=120
    h2 = relu(W2' h1 + b2')                    (BN1 folded)           M=60
    h3 = relu(W3' h2 + b3')                    (BN2 folded)           M=28
    phi = W4'' h3          (duplicated rows: cos block & sin block)   M=2*nf
    feats = Sin(phi + b4'')   one ACT op (cos via +pi/2 bias)
    h5 = relu(W5'' feats + b5'')               (harmonic coeffs folded) M=56
    h6 = relu(W6' h5 + b6')                    (BN5 folded)           M=112
    h7 = relu(W7' h6 + b7')                    (BN6 folded)           M=56
    y  = W8 h7 + b8                                                   M=36

  The quantum block is EXACT as a 41-frequency trigonometric polynomial:
  z_i(theta) = sum_f a_if cos(f.theta) + b_if sin(f.theta), f in {0,+-1}^4
  (canonical half-space).  Coefficients are fit at runtime from params['qw']
  by least squares in float64 (residual ~1e-15).

* x is transposed host-side per shard so all DMA is contiguous; matmuls use
  float32r (full-rate fp32 streaming on the PE).
"""

import sys

sys.path.insert(0, "/opt/trn_rl_repo")

import itertools
import math
from contextlib import ExitStack

import numpy as np

import concourse.bass as bass  # noqa: E402
import concourse.tile as tile  # noqa: E402
from concourse import bacc, mybir  # noqa: E402
from concourse.bass_utils import run_bass_kernel_spmd  # noqa: E402

F32 = mybir.dt.float32
F32R = mybir.dt.float32r
AF = mybir.ActivationFunctionType
ALU = mybir.AluOpType

B_TOTAL = 131072
D_IN = 252
D_OUT = 36
N_CORES = 8
B_CORE = B_TOTAL // N_CORES  # 16384
SUPER = 2048                 # columns handled per DMA round
BT = 512                     # matmul free dim (one PSUM bank of fp32)

_CNOT_PAIRS = [(0, 1), (1, 2), (2, 3), (3, 0)]
_BN_EPS = 1e-5


# ----------------------------------------------------------------------------
# host-side math: quantum block -> trigonometric polynomial
# ----------------------------------------------------------------------------

def _quantum_np(xq, w):
    B = xq.shape[0]
    s = np.zeros((B, 16), dtype=np.complex128)
    s[:, 0] = 1.0
    s = s.reshape(B, 2, 2, 2, 2)

    def apply_1q(s, g, wire):
        s = np.moveaxis(s, wire + 1, -1)
        if g.ndim == 3:
            s = np.einsum("zab,z...b->z...a", g, s)
        else:
            s = np.einsum("ab,...b->...a", g, s)
        return np.moveaxis(s, -1, wire + 1)

    def apply_rz(s, phi, wire):
        f = np.stack([np.exp(-0.5j * phi), np.exp(0.5j * phi)])
        s = np.moveaxis(s, wire + 1, -1) * f
        return np.moveaxis(s, -1, wire + 1)

    def apply_cnot(s, c, t):
        s = np.moveaxis(s, [c + 1, t + 1], [-2, -1])
        s = np.stack([s[..., 0, :], s[..., 1, ::-1]], axis=-2)
        return np.moveaxis(s, [-2, -1], [c + 1, t + 1])

    half = xq * (np.pi / 4)
    c, si = np.cos(half), np.sin(half)
    for i in range(4):
        g = np.stack([np.stack([c[:, i], -si[:, i]], -1),
                      np.stack([si[:, i], c[:, i]], -1)], -2).astype(np.complex128)
        s = apply_1q(s, g, i)
    for l in range(2):
        for i in range(4):
            t = 0.5 * w[l, i, 0]
            ry = np.array([[np.cos(t), -np.sin(t)], [np.sin(t), np.cos(t)]],
                          dtype=np.complex128)
            s = apply_1q(s, ry, i)
            s = apply_rz(s, w[l, i, 1], i)
        for (cq, tq) in _CNOT_PAIRS:
            s = apply_cnot(s, cq, tq)
    p = s.real ** 2 + s.imag ** 2
    zs = []
    for i in range(4):
        axes = tuple(a for a in range(1, 5) if a != i + 1)
        m = p.sum(axis=axes)
        zs.append(m[:, 0] - m[:, 1])
    return np.stack(zs, axis=-1)


def _derive_harmonics(qw, trunc_tol=1e-7):
    """Fit z(theta) = Ccos cos(F theta) + Csin sin(F theta).

    Returns freqs [nf,4] (nonzero, truncated), Ccos/Csin [4,nf], c0 [4]
    (the zero-frequency constant).
    """
    freqs = []
    for f in itertools.product([-1, 0, 1], repeat=4):
        fa = np.array(f)
        nz = np.nonzero(fa)[0]
        if len(nz) == 0 or fa[nz[0]] > 0:
            freqs.append(fa)
    freqs = np.array(sorted(freqs, key=lambda f: (np.abs(f).sum(), list(f))),
                     dtype=np.float64)
    F = len(freqs)  # 41, zero freq included

    rng = np.random.RandomState(1234)
    theta = rng.uniform(-4, 4, size=(4096, 4))
    z = _quantum_np(theta * (2 / np.pi), np.asarray(qw, np.float64))
    X = np.concatenate([np.cos(theta @ freqs.T), np.sin(theta @ freqs.T)], axis=1)
    coef, *_ = np.linalg.lstsq(X, z, rcond=None)
    pred = X @ coef
    fit_err = np.abs(pred - z).max()
    assert fit_err < 1e-9, f"harmonic fit failed: {fit_err}"

    Ccos = coef[:F].T  # [4, F]
    Csin = coef[F:].T  # [4, F]

    zero_idx = int(np.where((freqs == 0).all(axis=1))[0][0])
    c0 = Ccos[:, zero_idx].copy()
    keep = [i for i in range(F) if i != zero_idx]
    mag = np.sqrt((Ccos ** 2).sum(0) + (Csin ** 2).sum(0))
    keep = [i for i in keep if mag[i] > trunc_tol]
    return freqs[keep], Ccos[:, keep], Csin[:, keep], c0


def _fold_params(params):
    """Fold BN layers + harmonic coefficients into a pure matmul chain.

    Returns dict of float64 lhsT weight blocks / bias vectors and nf.
    """
    p = {k: np.asarray(v, np.float64) for k, v in params.items()}
    freqs, Ccos, Csin, c0 = _derive_harmonics(p["qw"])
    nf = freqs.shape[0]

    g1 = p["g1"] / np.sqrt(1.0 + _BN_EPS)
    g2 = p["g2"] / np.sqrt(1.0 + _BN_EPS)
    g5 = p["g5"] / np.sqrt(1.0 + _BN_EPS)
    g6 = p["g6"] / np.sqrt(1.0 + _BN_EPS)

    W1, b1 = p["w1"], p["b1"]
    W2 = p["w2"] * g1[None, :]
    b2 = p["w2"] @ p["beta1"] + p["b2"]
    W3 = p["w3"] * g2[None, :]
    b3 = p["w3"] @ p["beta2"] + p["b3"]

    # phi rows: [cos-block; sin-block], each = (pi/2) F W4
    F_W4 = freqs @ p["w4"] * (np.pi / 2)        # [nf, 28]
    F_b4 = freqs @ p["b4"] * (np.pi / 2)        # [nf]
    W4d = np.concatenate([F_W4, F_W4], axis=0)  # [2nf, 28]
    b4d = np.concatenate([F_b4 + np.pi / 2, F_b4])  # cos rows first (+pi/2)

    # h5 = relu(W5 z + b5), z = Ccos ct + Csin st + c0
    W5e = np.concatenate([p["w5"] @ Ccos, p["w5"] @ Csin], axis=1)  # [56, 2nf]
    b5e = p["b5"] + p["w5"] @ c0

    W6 = p["w6"] * g5[None, :]
    b6 = p["w6"] @ p["beta5"] + p["b6"]
    W7 = p["w7"] * g6[None, :]
    b7 = p["w7"] @ p["beta6"] + p["b7"]
    W8, b8 = p["w8"], p["b8"]

    return {
        "W1": W1, "b1": b1, "W2": W2, "b2": b2, "W3": W3, "b3": b3,
        "W4d": W4d, "b4d": b4d, "W5e": W5e, "b5e": b5e,
        "W6": W6, "b6": b6, "W7": W7, "b7": b7, "W8": W8, "b8": b8,
        "nf": nf,
    }


def _host_forward(x, fp):
    """Numpy forward pass of the folded network (for phi-range check & debug)."""
    relu = lambda v: np.maximum(v, 0.0)
    h = relu(x @ fp["W1"].T + fp["b1"])
    h = relu(h @ fp["W2"].T + fp["b2"])
    h = relu(h @ fp["W3"].T + fp["b3"])
    phi = h @ fp["W4d"].T + fp["b4d"]
    feats = np.sin(phi)
    h = relu(feats @ fp["W5e"].T + fp["b5e"])
    h = relu(h @ fp["W6"].T + fp["b6"])
    h = relu(h @ fp["W7"].T + fp["b7"])
    return h @ fp["W8"].T + fp["b8"], phi


# ----------------------------------------------------------------------------
# weight packing
# ----------------------------------------------------------------------------

def _round_f32r(a):
    """Round fp32 to the TRN2 FP32R format (1s + 8e + 11m, RNE)."""
    u = np.ascontiguousarray(a, np.float32).view(np.uint32).astype(np.uint64)
    bias = 0x7FF + ((u >> 12) & 1)
    u = (u + bias) & 0xFFFFF000
    return u.astype(np.uint32).view(np.float32)


def _pack_weights(fp):
    nf = fp["nf"]
    nf2 = 2 * nf

    blocks = [  # (K, M, lhsT [K, M])
        ("w1a", fp["W1"][:, :128].T),            # [128, 120]
        ("w1b", fp["W1"][:, 128:].T),            # [124, 120]
        ("w2", fp["W2"].T),                      # [120, 60]
        ("w3", fp["W3"].T),                      # [60, 28]
        ("w4d", fp["W4d"].T),                    # [28, 2nf]
        ("w5", fp["W5e"].T),                     # [2nf, 56]
        ("w6", fp["W6"].T),                      # [56, 112]
        ("w7", fp["W7"].T),                      # [112, 56]
        ("w8", fp["W8"].T),                      # [56, 36]
    ]
    offs = {}
    col = 0
    for name, blk in blocks:
        offs[name] = (col, blk.shape[0], blk.shape[1])
        col += blk.shape[1]
    NW = col
    wpack = np.zeros((128, NW), np.float32)
    for name, blk in blocks:
        c0, K, M = offs[name]
        wpack[:K, c0:c0 + M] = blk.astype(np.float32)

    biases = [("b1", fp["b1"]), ("b2", fp["b2"]), ("b3", fp["b3"]),
              ("b4d", fp["b4d"]), ("b5", fp["b5e"]), ("b6", fp["b6"]),
              ("b7", fp["b7"]), ("b8", fp["b8"])]
    bpack = np.zeros((128, len(biases)), np.float32)
    boffs = {}
    for i, (name, b) in enumerate(biases):
        bpack[:len(b), i] = b.astype(np.float32)
        boffs[name] = i
    return wpack, bpack, offs, boffs, nf2


# ----------------------------------------------------------------------------
# bass program
# ----------------------------------------------------------------------------

def build_program(nf2, n_cols=B_CORE, offs=None, boffs=None, NW=None, repeat=1):
    """Build the SPMD Bass program.  n_cols = batch rows per core.

    repeat > 1 re-runs the whole pass inside one NEFF (for timing)."""
    assert n_cols % SUPER == 0
    n_super = n_cols // SUPER
    n_sub = SUPER // BT

    nc = bacc.Bacc("TRN2", target_bir_lowering=False, debug=False,
                   enable_asserts=False, num_devices=N_CORES)

    xt = nc.dram_tensor("xt", [D_IN, n_cols], F32R, kind="ExternalInput").ap()
    wp = nc.dram_tensor("wpack", [128, NW], F32R, kind="ExternalInput").ap()
    bp = nc.dram_tensor("bpack", [128, 8], F32, kind="ExternalInput").ap()
    yt = nc.dram_tensor("yt", [D_OUT, n_cols], F32, kind="ExternalOutput").ap()

    def W(name):
        c0, K, M = offs[name]
        return None, c0, K, M

    with ExitStack() as ctx:
        tc = ctx.enter_context(tile.TileContext(nc))
        wpool = ctx.enter_context(tc.tile_pool(name="w", bufs=1))
        xpool = ctx.enter_context(tc.tile_pool(name="x", bufs=3))
        hpool = ctx.enter_context(tc.tile_pool(name="h", bufs=2))
        ypool = ctx.enter_context(tc.tile_pool(name="y", bufs=2))
        pspool = ctx.enter_context(tc.tile_pool(name="ps", bufs=6, space="PSUM"))

        wsb = wpool.tile([128, NW], F32R)
        nc.sync.dma_start(wsb[:], wp[:])
        bsb = wpool.tile([128, 8], F32)
        nc.sync.dma_start(bsb[:], bp[:])

        def wap(name):
            c0, K, M = offs[name]
            return wsb[0:K, c0:c0 + M]

        def bias(name, P):
            i = boffs[name]
            return bsb[0:P, i:i + 1]

        def mm(pool_tag, lhs_name, rhs_ap, extra=None):
            c0, K, M = offs[lhs_name]
            ps = pspool.tile([M, BT], F32, tag="ps")
            nc.tensor.matmul(ps[:], wap(lhs_name), rhs_ap,
                             start=True, stop=(extra is None))
            if extra is not None:
                nc.tensor.matmul(ps[:], wap(extra[0]), extra[1],
                                 start=False, stop=True)
            return ps

        def act_relu(ps, bname, P):
            h = hpool.tile([P, BT], F32R, tag=bname)
            nc.scalar.activation(h[:], ps[:], AF.Relu, bias=bias(bname, P))
            return h

        def dve_relu(ps, bname, P):
            h = hpool.tile([P, BT], F32R, tag=bname)
            nc.vector.tensor_scalar(h[:], ps[:], bias(bname, P), 0.0,
                                    op0=ALU.add, op1=ALU.max)
            return h

        for s in range(n_super * repeat):
            s = s % n_super
            cs = s * SUPER
            xa = xpool.tile([128, SUPER], F32R, tag="xa")
            nc.sync.dma_start(xa[:], xt[0:128, cs:cs + SUPER])
            xb = xpool.tile([124, SUPER], F32R, tag="xb")
            nc.sync.dma_start(xb[:], xt[128:252, cs:cs + SUPER])
            yb = ypool.tile([D_OUT, SUPER], F32, tag="y")

            for j in range(n_sub):
                c = slice(j * BT, (j + 1) * BT)
                ps1 = mm("ps", "w1a", xa[:, c], extra=("w1b", xb[:, c]))
                h1 = act_relu(ps1, "b1", 120)
                ps2 = mm("ps", "w2", h1[:])
                h2 = dve_relu(ps2, "b2", 60)
                ps3 = mm("ps", "w3", h2[:])
                h3 = dve_relu(ps3, "b3", 28)
                ps4 = mm("ps", "w4d", h3[:])
                ft = hpool.tile([nf2, BT], F32R, tag="ft")
                nc.scalar.activation(ft[:], ps4[:], AF.Sin, bias=bias("b4d", nf2))
                ps5 = mm("ps", "w5", ft[:])
                h5 = dve_relu(ps5, "b5", 56)
                ps6 = mm("ps", "w6", h5[:])
                h6 = act_relu(ps6, "b6", 112)
                ps7 = mm("ps", "w7", h6[:])
                h7 = dve_relu(ps7, "b7", 56)
                ps8 = mm("ps", "w8", h7[:])
                nc.vector.tensor_scalar(yb[:, c], ps8[:], bias("b8", D_OUT), None,
                                        op0=ALU.add)

            nc.sync.dma_start(yt[:, cs:cs + SUPER], yb[:])

    nc.compile()
    return nc


# ----------------------------------------------------------------------------
# entry point
# ----------------------------------------------------------------------------

_PROG_CACHE = {}
LAST_EXEC_TIME_NS = None
LAST_RESULTS = None


def _get_program(nf2, n_cols, offs_key, offs, boffs, NW):
    key = (nf2, n_cols, offs_key)
    if key not in _PROG_CACHE:
        _PROG_CACHE[key] = build_program(nf2, n_cols, offs, boffs, NW)
    return _PROG_CACHE[key]


def kernel(x, params, trace=False):
    global LAST_EXEC_TIME_NS, LAST_RESULTS
    x = np.asarray(x, np.float32)
    assert x.shape == (B_TOTAL, D_IN), x.shape

    fp = _fold_params(params)
    wpack, bpack, offs, boffs, nf2 = _pack_weights(fp)
    NW = wpack.shape[1]

    # range guard for the Sin LUT ([-pi, pi] on the scalar engine): compute
    # exact phi range host-side (cheap BLAS pass over the folded fp32 net)
    fp32 = {k: (np.asarray(v, np.float32) if isinstance(v, np.ndarray) else v)
            for k, v in fp.items()}
    _, phi = _host_forward(x, fp32)
    phi_max = float(np.abs(phi).max())
    assert phi_max < 3.1, (
        f"phi range {phi_max} exceeds scalar-engine Sin domain; "
        "mod-based range reduction required")

    offs_key = tuple(sorted((k, v[0], v[1], v[2]) for k, v in offs.items()))
    nc = _get_program(nf2, B_CORE, offs_key, offs, boffs, NW)

    wpack = _round_f32r(wpack)
    in_maps = []
    for c in range(N_CORES):
        xs = _round_f32r(np.ascontiguousarray(x[c * B_CORE:(c + 1) * B_CORE].T))
        in_maps.append({"xt": xs, "wpack": wpack, "bpack": bpack})

    res = run_bass_kernel_spmd(nc, in_maps, core_ids=list(range(N_CORES)),
                               trace=trace)
    LAST_EXEC_TIME_NS = res.exec_time_ns
    LAST_RESULTS = res

    y = np.empty((B_TOTAL, D_OUT), np.float32)
    for c in range(N_CORES):
        y[c * B_CORE:(c + 1) * B_CORE] = res.results[c]["yt"].T
    return y
